# revision 7
# baseline (speedup 1.0000x reference)
"""Trainium2 Bass kernel for nn_Attention_5420248728069.

Data-parallel over 8 NeuronCores on v_code rows; obs_code and weights
replicated; no collectives.

    v_value   = v @ Wv.T ; obs_value = obs @ Wv.T
    v_query   = v @ Wq.T ; v_key = v @ Wk.T ; obs_key = obs @ Wk.T
    S         = v_query @ obs_key.T            # cross attention [N, M]
    s_self    = rowsum(v_query * v_key)        # [N]
    w         = softmax(concat([s_self, S]) / sqrt(E))
    out       = LayerNorm(w0 * v_value + w[:,1:] @ obs_value + v) * gamma + beta

Algebraic refactoring (exact in f32):
    A   = Wq.T @ Wk                            # [E, E], computed once
    S.T = ((v @ A) @ obs.T).T ;  s_self = rowsum((v@A) * v)
    y   = (w0 * v + expS @ obs) @ Wv.T / Z + v # unnormalized-softmax form

Precision: attention matmuls (scores, partition function, weighted sum) and
the projections run in fp8e4 DoubleRow (contraction 256/matmul) with a
constant logit shift of -4 so exp() fits fp8 range; epilogue (residual +
LayerNorm) in f32.

v2 structure (vs the 193us baseline):
  - obs.T produced by DMA XBAR transposes (bf16) instead of 128 TensorE
    transpose matmuls: obs f32 load -> bf16 cast -> dma_start(transpose=True)
    -> fp8 cast, pipelined per 512-row group under the main loop.
  - big-descriptor DMA loads (8-16KB per partition) split across the two
    HWDGE queues (scalar: wq, v, obs even groups; sync: wk, wv, obs odd).
  - A computed in fp8 DoubleRow; one exp per t-step over [128,1024] PSUM.
  - main loop emits score(t) then Z/uT(t-1) so TensorE covers exp latency;
    epilogue of block 0 overlaps block 1's loop.
"""

import numpy as np

N_GLOBAL = 8192
M = 4096
E = 512
CORES = 8
NLOC = N_GLOBAL // CORES  # 1024
TEMPERATURE = 22.627416997969522  # sqrt(E)
EPS = 1e-6
P = 128

_CACHED_NC = None


def _build():
    from contextlib import ExitStack

    import concourse.bass as bass
    import concourse.tile as tile
    from concourse import bacc, mybir
    from concourse.masks import make_identity

    f32 = mybir.dt.float32
    bf16 = mybir.dt.bfloat16
    f8 = mybir.dt.float8e4
    DR = mybir.MatmulPerfMode.DoubleRow
    SHIFT = 4.0  # softmax logit shift so exp() fits fp8e4 range
    AF = mybir.ActivationFunctionType
    ALU = mybir.AluOpType

    nc = bacc.Bacc("TRN2", target_bir_lowering=False, debug=False)

    v_d = nc.dram_tensor("v_code", [NLOC, E], f32, kind="ExternalInput")
    obs_d = nc.dram_tensor("obs_code", [M, E], f32, kind="ExternalInput")
    wq_d = nc.dram_tensor("Wq", [E, E], f32, kind="ExternalInput")
    wk_d = nc.dram_tensor("Wk", [E, E], f32, kind="ExternalInput")
    wv_d = nc.dram_tensor("Wv", [E, E], f32, kind="ExternalInput")
    gamma_d = nc.dram_tensor("gamma", [E], f32, kind="ExternalInput")
    beta_d = nc.dram_tensor("beta", [E], f32, kind="ExternalInput")
    out_d = nc.dram_tensor("out", [NLOC, E], f32, kind="ExternalOutput")

    def bcast_ap(ap_1row, parts=P):
        # replicate a [1, F] (or [F]) DRAM AP across `parts` partitions
        dims = [list(d) for d in ap_1row.ap]
        if len(dims) > 1 and dims[0][1] == 1:
            dims = dims[1:]
        return bass.AP(
            tensor=ap_1row.tensor, offset=ap_1row.offset, ap=[[0, parts]] + dims
        )

    with tile.TileContext(nc) as tc, ExitStack() as ctx:
        const = ctx.enter_context(tc.tile_pool(name="const", bufs=1))
        persist = ctx.enter_context(tc.tile_pool(name="persist", bufs=1))
        dram = ctx.enter_context(tc.tile_pool(name="dram", bufs=1, space="DRAM"))
        # obs streaming pools
        ofp = ctx.enter_context(tc.tile_pool(name="ofp", bufs=3))
        obp = ctx.enter_context(tc.tile_pool(name="obp", bufs=3))
        otbp = ctx.enter_context(tc.tile_pool(name="otbp", bufs=3))
        expp = ctx.enter_context(tc.tile_pool(name="expp", bufs=3))
        prodp = ctx.enter_context(tc.tile_pool(name="prodp", bufs=2))
        epi = ctx.enter_context(tc.tile_pool(name="epi", bufs=3))

        # ---- persistent SBUF tensors
        v_f32 = persist.tile([P, 8, E], f32, tag="v_f32")
        vT = persist.tile([P, 4, NLOC], f8, tag="vT")
        vAT = persist.tile([P, 4, NLOC], f8, tag="vAT")
        A_sb = persist.tile([P, 4, E], f8, tag="A")
        WvT = persist.tile([P, 4, E], f8, tag="WvT")
        obs_f8 = persist.tile([P, 32, E], f8, tag="obs_f8")
        # obs.T fp8, mc-major: [e_lo, mc, e_hi, p]; column (mc*128+p) of the
        # logical [E, M] transpose holds obs row (32p + mc); e = e_hi*128+e_lo
        obsT = persist.tile([P, 32, 4, P], f8, tag="obsT")
        uT = persist.tile([P, 4, NLOC], f8, tag="uT")
        w0 = persist.tile([P, 8], f32, tag="w0")
        w0_bc = persist.tile([P, NLOC], bf16, tag="w0_bc")
        w0v = persist.tile([P, 4, NLOC], bf16, tag="w0v")
        ztok = persist.tile([P, 8], f32, tag="ztok")
        recipZ = persist.tile([P, 8], f32, tag="recipZ")

        gamma_b = const.tile([P, E], f32, tag="gamma")
        beta_b = const.tile([P, E], f32, tag="beta")
        identity = const.tile([P, P], f32, tag="ident")
        ones_bf = const.tile([P, 1], bf16, tag="ones")
        ones_f8z = const.tile([P, 2, P], f8, tag="ones8z")
        eps_t = const.tile([P, 1], f32, tag="eps")
        nshift_t = const.tile([P, 1], f32, tag="nshift")

        make_identity(nc, identity)
        nc.vector.memset(ones_bf, 1.0)
        nc.vector.memset(ones_f8z, 1.0)
        nc.vector.memset(eps_t, EPS)
        nc.vector.memset(nshift_t, -SHIFT)
        nc.gpsimd.dma_start(out=gamma_b, in_=bcast_ap(gamma_d.ap()))
        nc.gpsimd.dma_start(out=beta_b, in_=bcast_ap(beta_d.ap()))

        scr_z = dram.tile([1, NLOC], f32, tag="scr_z")
        scr_w0 = dram.tile([1, NLOC], f32, tag="scr_w0")

        # token n <-> (p, c) mapping is n = 8p + c (partition-major loads
        # give contiguous 16KB DMA descriptors); the output write uses the
        # same mapping, so results land in canonical row order.
        out_r = out_d.ap().rearrange("(p c) e -> p c e", c=8)

        # ---- setup phase (own pools, freed before the main-loop PSUM pools)
        sctx = ctx.enter_context(ExitStack())
        stage = sctx.enter_context(tc.tile_pool(name="stage", bufs=1))
        ps_setup = sctx.enter_context(tc.tile_pool(name="ps_set", bufs=2, space="PSUM"))

        # ---- loads. scalar HWDGE: wq, v, obs even groups; sync HWDGE:
        # wk, wv, obs odd groups. All with large per-partition descriptors.
        wq_f = stage.tile([P, 4, E], f32, tag="wq_f")
        nc.scalar.dma_start(wq_f, wq_d.ap().rearrange("(p c) e -> p c e", c=4))
        v_r = v_d.ap().rearrange("(p c) e -> p c e", c=8)
        nc.scalar.dma_start(v_f32, v_r)
        wk_f = stage.tile([P, 4, E], f32, tag="wk_f")
        nc.sync.dma_start(wk_f, wk_d.ap().rearrange("(p c) e -> p c e", c=4))
        wv_f = stage.tile([P, 4, E], f32, tag="wv_f")
        nc.sync.dma_start(wv_f, wv_d.ap().rearrange("(c p) e -> p c e", p=P))

        obs_r = obs_d.ap().rearrange("(p c) e -> p c e", c=32)
        obs_stage = []
        for g in range(8):
            of = ofp.tile([P, 4, E], f32, tag="obs_f")
            eng = nc.scalar if g % 2 == 0 else nc.sync
            eng.dma_start(of, obs_r[:, g * 4 : (g + 1) * 4, :])
            obs_stage.append(of)

        # ---- A = Wq.T @ Wk in fp8 DoubleRow (scale folded into exp)
        # (casts on DVE so ScalarE's act-table loads don't delay A)
        wq8 = stage.tile([P, 4, E], f8, tag="wq8")
        nc.vector.tensor_copy(wq8, wq_f)
        wk8 = stage.tile([P, 4, E], f8, tag="wk8")
        nc.vector.tensor_copy(wk8, wk_f)
        for ic in range(4):
            psA = ps_setup.tile([P, E], f32, tag="psA")
            for u in range(2):
                nc.tensor.matmul(
                    psA,
                    lhsT=wq8[:, 2 * u : 2 * u + 2, ic * P : (ic + 1) * P],
                    rhs=wk8[:, 2 * u : 2 * u + 2, :],
                    start=(u == 0),
                    stop=(u == 1),
                    perf_mode=DR,
                )
            nc.scalar.copy(A_sb[:, ic, :], psA)

        # ---- vT via TensorE transposes (f32 in, cast to f8 on copy-out)
        for ec in range(4):
            for g in range(2):
                pst = ps_setup.tile([P, 4 * P], f32, tag="pst")
                for j in range(4):
                    nk = g * 4 + j
                    nc.tensor.transpose(
                        pst[:, j * P : (j + 1) * P],
                        v_f32[:, nk, ec * P : (ec + 1) * P],
                        identity,
                    )
                nc.vector.tensor_copy(vT[:, ec, g * 512 : (g + 1) * 512], pst)

        # ---- vAT = (v @ A).T   [e2, n]  (fp8)
        for e2 in range(4):
            for nb in range(2):
                psv = ps_setup.tile([P, 512], f32, tag="psv")
                for u in range(2):
                    nc.tensor.matmul(
                        psv,
                        lhsT=A_sb[:, 2 * u : 2 * u + 2, e2 * P : (e2 + 1) * P],
                        rhs=vT[:, 2 * u : 2 * u + 2, nb * 512 : (nb + 1) * 512],
                        start=(u == 0),
                        stop=(u == 1),
                        perf_mode=DR,
                    )
                nc.vector.tensor_copy(vAT[:, e2, nb * 512 : (nb + 1) * 512], psv)

        # ---- WvT via TensorE transposes (only needed by the epilogue)
        for jc in range(4):  # e_in slice -> WvT partition chunk
            pst = ps_setup.tile([P, 4 * P], f32, tag="pst")
            for ic in range(4):  # e_out chunk
                nc.tensor.transpose(
                    pst[:, ic * P : (ic + 1) * P],
                    wv_f[:, ic, jc * P : (jc + 1) * P],
                    identity,
                )
            nc.scalar.copy(WvT[:, jc, :], pst)

        # ---- self score (token-major [n,1] per chunk) and w0 = exp(.-S)
        ps_sf = ps_setup.tile([P, 8], f32, tag="ps_sf")
        for ec in range(4):
            prod_ec = prodp.tile([P, NLOC], bf16, tag="prod")
            nc.vector.tensor_mul(prod_ec, vAT[:, ec, :], vT[:, ec, :])
            for nk in range(8):
                nc.tensor.matmul(
                    ps_sf[:, nk : nk + 1],
                    lhsT=prod_ec[:, nk * P : (nk + 1) * P],
                    rhs=ones_bf,
                    start=(ec == 0),
                    stop=(ec == 3),
                )
        nc.scalar.activation(
            w0, ps_sf, AF.Exp, bias=nshift_t, scale=1.0 / TEMPERATURE
        )
        # w0 row-major broadcast [P, NLOC] via DRAM roundtrip (GpSimd)
        nc.gpsimd.dma_start(scr_w0.rearrange("o (a p) -> (o p) a", p=P), w0)
        nc.gpsimd.dma_start(w0_bc, bcast_ap(scr_w0[:]))

        # ---- obs pipeline: f32 -> bf16 cast -> DMA XBAR transpose -> f8.
        # bf16 casts: DVE for the early groups, gpsimd for the last two
        # (slow but off the critical chain). m-major f8 casts split
        # scalar/gpsimd. Transposes alternate between the two HWDGE queues.
        for g in range(8):
            of = obs_stage[g]
            if g % 2 == 0:
                nc.scalar.copy(obs_f8[:, g * 4 : (g + 1) * 4, :], of)
            else:
                nc.gpsimd.tensor_copy(obs_f8[:, g * 4 : (g + 1) * 4, :], of)
            ob = obp.tile([P, 4, E], bf16, tag="obs_bf")
            if g >= 6:
                nc.gpsimd.tensor_copy(ob, of)
            else:
                nc.vector.tensor_copy(ob, of)
            otb = otbp.tile([P, 4, 4, P], bf16, tag="obsT_bf")
            teng = nc.sync if g % 2 == 0 else nc.scalar
            teng.dma_start(otb, ob, transpose=True)
            # cast to f8 (dense 16-bit read, fast on DVE)
            nc.vector.tensor_copy(obsT[:, g * 4 : (g + 1) * 4, :, :], otb)

        # ---- w0 * v.T term, precomputed for the uT drains
        for ec in range(4):
            nc.vector.tensor_mul(w0v[:, ec, :], vT[:, ec, :], w0_bc)

        sctx.close()
        ps_s_pool = ctx.enter_context(tc.tile_pool(name="ps_s", bufs=1, space="PSUM"))
        ps_ut_pool = ctx.enter_context(tc.tile_pool(name="ps_ut", bufs=1, space="PSUM"))
        ps_z_pool = ctx.enter_context(tc.tile_pool(name="ps_z", bufs=1, space="PSUM"))
        ps_y_pool = ctx.enter_context(tc.tile_pool(name="ps_y", bufs=1, space="PSUM"))

        # ---- main loop: 2 token blocks of 512; 16 obs chunks of 256 each.
        # Emit score(t) then Z/uT(t-1): TensorE does uT work while ScalarE
        # runs exp(t), and ps_s (single-buffered, 2 banks) frees in time.
        def make_epilogue_chunks(nb):
            # returns 4 closures, one per token chunk; first also emits the
            # recipZ prep. ps_y is drained to SBUF by a fast scalar copy so
            # TensorE never waits on the DVE LayerNorm chain.
            def chunk(i):
                def emit():
                    nk = nb * 4 + i
                    if i == 0:
                        c4 = slice(nb * 4, (nb + 1) * 4)
                        nc.vector.tensor_add(ztok[:, c4], ztok[:, c4], w0[:, c4])
                        nc.vector.reciprocal(recipZ[:, c4], ztok[:, c4])
                    ps_y = ps_y_pool.tile([P, E], f32, tag="y")
                    for u in range(2):
                        nc.tensor.matmul(
                            ps_y,
                            lhsT=uT[:, 2 * u : 2 * u + 2, nk * P : (nk + 1) * P],
                            rhs=WvT[:, 2 * u : 2 * u + 2, :],
                            start=(u == 0),
                            stop=(u == 1),
                            perf_mode=DR,
                        )
                    y_sb = epi.tile([P, E], f32, tag="y_sb")
                    nc.scalar.copy(y_sb, ps_y)
                    y2 = epi.tile([P, E], f32, tag="y2")
                    nc.vector.scalar_tensor_tensor(
                        y2,
                        in0=y_sb,
                        scalar=recipZ[:, nk : nk + 1],
                        in1=v_f32[:, nk, :],
                        op0=ALU.mult,
                        op1=ALU.add,
                    )
                    stats = epi.tile([P, 6], f32, tag="stats")
                    nc.vector.bn_stats(stats, y2)
                    mv = epi.tile([P, 2], f32, tag="mv")
                    nc.vector.bn_aggr(mv, stats)
                    std = epi.tile([P, 1], f32, tag="std")
                    nc.scalar.activation(std, mv[:, 1:2], AF.Sqrt, bias=eps_t)
                    rstd = epi.tile([P, 1], f32, tag="rstd")
                    nc.vector.reciprocal(rstd, std)
                    nc.vector.tensor_scalar(
                        y2,
                        in0=y2,
                        scalar1=mv[:, 0:1],
                        scalar2=rstd,
                        op0=ALU.subtract,
                        op1=ALU.mult,
                    )
                    nc.vector.tensor_mul(y2, y2, gamma_b)
                    nc.vector.tensor_add(y2, y2, beta_b)
                    nc.sync.dma_start(out_r[:, nk, :], y2)
                return emit
            return [chunk(i) for i in range(4)]

        pending_epilogue = None
        for nb in range(2):
            nsl = slice(nb * 512, (nb + 1) * 512)
            ps_uT = ps_ut_pool.tile([P, 4, 512], f32, tag="uT")
            ps_z = ps_z_pool.tile([P, 512], f32, tag="z")
            pend = None  # (t, ex2) whose Z/uT matmuls are not yet emitted

            def flush(t, ex2):
                nc.tensor.matmul(
                    ps_z,
                    lhsT=ones_f8z,
                    rhs=ex2,
                    start=(t == 0),
                    stop=(t == 15),
                    perf_mode=DR,
                )
                for es in range(4):
                    nc.tensor.matmul(
                        ps_uT[:, es, :],
                        lhsT=obs_f8[:, 2 * t : 2 * t + 2, es * P : (es + 1) * P],
                        rhs=ex2,
                        start=(t == 0),
                        stop=(t == 15),
                        perf_mode=DR,
                    )

            for t in range(16):
                ps_s = ps_s_pool.tile([P, 2, 512], f32, tag="s")
                for j in range(2):
                    mc = 2 * t + j
                    for u in range(2):
                        nc.tensor.matmul(
                            ps_s[:, j, :],
                            lhsT=obsT[:, mc, 2 * u : 2 * u + 2, :],
                            rhs=vAT[:, 2 * u : 2 * u + 2, nsl],
                            start=(u == 0),
                            stop=(u == 1),
                            perf_mode=DR,
                        )
                if pending_epilogue and t < 4:
                    # overlap block 0's epilogue with block 1's first steps
                    pending_epilogue[t]()
                if pend is not None:
                    flush(*pend)
                ex2 = expp.tile([P, 2, 512], f8, tag="ex")
                nc.scalar.activation(
                    ex2, ps_s, AF.Exp, bias=nshift_t, scale=1.0 / TEMPERATURE
                )
                pend = (t, ex2)
            flush(*pend)

            # drain Z first (epilogue roundtrip starts while uT drains)
            zrow = epi.tile([1, 512], f32, tag="zrow")
            nc.vector.tensor_copy(zrow, ps_z[0:1, :])
            nc.sync.dma_start(scr_z[:, nsl], zrow)
            nc.sync.dma_start(
                ztok[:, nb * 4 : (nb + 1) * 4],
                scr_z[:, nsl].rearrange("o (a p) -> (o p) a", p=P),
            )
            # drain uT (+ fold in w0 * v term)
            for ec in range(4):
                nc.vector.tensor_add(
                    uT[:, ec, nsl], w0v[:, ec, nsl], ps_uT[:, ec, :]
                )
            if nb == 0:
                pending_epilogue = make_epilogue_chunks(0)
            else:
                for step in make_epilogue_chunks(1):
                    step()

    nc.compile()
    return nc


def _get_nc():
    global _CACHED_NC
    if _CACHED_NC is None:
        _CACHED_NC = _build()
    return _CACHED_NC


def _in_maps(v_code, obs_code, Wq, Wk, Wv, gamma, beta):
    def f(x):
        return np.ascontiguousarray(np.asarray(x), dtype=np.float32)

    shared = {
        "obs_code": f(obs_code),
        "Wq": f(Wq),
        "Wk": f(Wk),
        "Wv": f(Wv),
        "gamma": f(gamma),
        "beta": f(beta),
    }
    return [
        {"v_code": f(v_code[c * NLOC : (c + 1) * NLOC]), **shared}
        for c in range(CORES)
    ]


def run(trace=False, **inputs):
    from concourse.bass_utils import run_bass_kernel_spmd

    nc = _get_nc()
    res = run_bass_kernel_spmd(
        nc, _in_maps(**inputs), core_ids=list(range(CORES)), trace=trace
    )
    out = np.concatenate(
        [res.results[c]["out"] for c in range(CORES)], axis=0
    ).astype(np.float32)
    return out, res


def kernel(**inputs) -> np.ndarray:
    out, _ = run(trace=False, **inputs)
    return out


# revision 8
# speedup vs baseline: 1.1185x; 1.1185x over previous
"""Trainium2 Bass kernel for nn_Attention_5420248728069.

Data-parallel over 8 NeuronCores on v_code rows; obs_code and weights
replicated; no collectives.

    v_value   = v @ Wv.T ; obs_value = obs @ Wv.T
    v_query   = v @ Wq.T ; v_key = v @ Wk.T ; obs_key = obs @ Wk.T
    S         = v_query @ obs_key.T            # cross attention [N, M]
    s_self    = rowsum(v_query * v_key)        # [N]
    w         = softmax(concat([s_self, S]) / sqrt(E))
    out       = LayerNorm(w0 * v_value + w[:,1:] @ obs_value + v) * gamma + beta

Algebraic refactoring (exact in f32):
    A   = Wq.T @ Wk                            # [E, E], computed once
    S.T = ((v @ A) @ obs.T).T ;  s_self = rowsum((v@A) * v)
    y   = (w0 * v + expS @ obs) @ Wv.T / Z + v # unnormalized-softmax form

Precision: attention matmuls (scores, partition function, weighted sum) and
the projections run in fp8e4 DoubleRow (contraction 256/matmul) with a
constant logit shift of -4 so exp() fits fp8 range; epilogue (residual +
LayerNorm) in f32.

v2 structure (vs the 193us baseline):
  - obs.T produced by DMA XBAR transposes (bf16) instead of 128 TensorE
    transpose matmuls: obs f32 load -> bf16 cast -> dma_start(transpose=True)
    -> fp8 cast, pipelined per 512-row group under the main loop.
  - big-descriptor DMA loads (8-16KB per partition) split across the two
    HWDGE queues (scalar: wq, v, obs even groups; sync: wk, wv, obs odd).
  - A computed in fp8 DoubleRow; one exp per t-step over [128,1024] PSUM.
  - main loop emits score(t) then Z/uT(t-1) so TensorE covers exp latency;
    epilogue of block 0 overlaps block 1's loop.
"""

import numpy as np

N_GLOBAL = 8192
M = 4096
E = 512
CORES = 8
NLOC = N_GLOBAL // CORES  # 1024
TEMPERATURE = 22.627416997969522  # sqrt(E)
EPS = 1e-6
P = 128

_CACHED_NC = None


def _build():
    from contextlib import ExitStack

    import concourse.bass as bass
    import concourse.tile as tile
    from concourse import bacc, mybir
    from concourse.masks import make_identity

    f32 = mybir.dt.float32
    bf16 = mybir.dt.bfloat16
    f8 = mybir.dt.float8e4
    DR = mybir.MatmulPerfMode.DoubleRow
    SHIFT = 4.0  # softmax logit shift so exp() fits fp8e4 range
    AF = mybir.ActivationFunctionType
    ALU = mybir.AluOpType

    nc = bacc.Bacc("TRN2", target_bir_lowering=False, debug=False)

    v_d = nc.dram_tensor("v_code", [NLOC, E], f32, kind="ExternalInput")
    obs_d = nc.dram_tensor("obs_code", [M, E], f32, kind="ExternalInput")
    wq_d = nc.dram_tensor("Wq", [E, E], f32, kind="ExternalInput")
    wk_d = nc.dram_tensor("Wk", [E, E], f32, kind="ExternalInput")
    wv_d = nc.dram_tensor("Wv", [E, E], f32, kind="ExternalInput")
    gamma_d = nc.dram_tensor("gamma", [E], f32, kind="ExternalInput")
    beta_d = nc.dram_tensor("beta", [E], f32, kind="ExternalInput")
    out_d = nc.dram_tensor("out", [NLOC, E], f32, kind="ExternalOutput")

    def bcast_ap(ap_1row, parts=P):
        # replicate a [1, F] (or [F]) DRAM AP across `parts` partitions
        dims = [list(d) for d in ap_1row.ap]
        if len(dims) > 1 and dims[0][1] == 1:
            dims = dims[1:]
        return bass.AP(
            tensor=ap_1row.tensor, offset=ap_1row.offset, ap=[[0, parts]] + dims
        )

    with tile.TileContext(nc) as tc, ExitStack() as ctx:
        const = ctx.enter_context(tc.tile_pool(name="const", bufs=1))
        persist = ctx.enter_context(tc.tile_pool(name="persist", bufs=1))
        dram = ctx.enter_context(tc.tile_pool(name="dram", bufs=1, space="DRAM"))
        # obs streaming pools
        ofp = ctx.enter_context(tc.tile_pool(name="ofp", bufs=3))
        obp = ctx.enter_context(tc.tile_pool(name="obp", bufs=3))
        otbp = ctx.enter_context(tc.tile_pool(name="otbp", bufs=3))
        expp = ctx.enter_context(tc.tile_pool(name="expp", bufs=3))
        prodp = ctx.enter_context(tc.tile_pool(name="prodp", bufs=2))
        epi = ctx.enter_context(tc.tile_pool(name="epi", bufs=3))

        # ---- persistent SBUF tensors
        v_f32 = persist.tile([P, 8, E], f32, tag="v_f32")
        vT = persist.tile([P, 4, NLOC], f8, tag="vT")
        vAT = persist.tile([P, 4, NLOC], f8, tag="vAT")
        A_sb = persist.tile([P, 4, E], f8, tag="A")
        WvT = persist.tile([P, 4, E], f8, tag="WvT")
        obs_f8 = persist.tile([P, 32, E], f8, tag="obs_f8")
        # obs.T fp8, mc-major: [e_lo, mc, e_hi, p]; column (mc*128+p) of the
        # logical [E, M] transpose holds obs row (32p + mc); e = e_hi*128+e_lo
        obsT = persist.tile([P, 32, 4, P], f8, tag="obsT")
        uT = persist.tile([P, 4, NLOC], f8, tag="uT")
        w0 = persist.tile([P, 8], f32, tag="w0")
        w0_bc = persist.tile([P, NLOC], bf16, tag="w0_bc")
        w0v = persist.tile([P, 4, NLOC], bf16, tag="w0v")
        ztok = persist.tile([P, 8], f32, tag="ztok")
        recipZ = persist.tile([P, 8], f32, tag="recipZ")

        gamma_b = const.tile([P, E], f32, tag="gamma")
        beta_b = const.tile([P, E], f32, tag="beta")
        identity = const.tile([P, P], f32, tag="ident")
        ones_bf = const.tile([P, 1], bf16, tag="ones")
        ones_f8z = const.tile([P, 2, P], f8, tag="ones8z")
        eps_t = const.tile([P, 1], f32, tag="eps")
        nshift_t = const.tile([P, 1], f32, tag="nshift")

        make_identity(nc, identity)
        nc.vector.memset(ones_bf, 1.0)
        nc.vector.memset(ones_f8z, 1.0)
        nc.vector.memset(eps_t, EPS)
        nc.vector.memset(nshift_t, -SHIFT)
        nc.gpsimd.dma_start(out=gamma_b, in_=bcast_ap(gamma_d.ap()))
        nc.gpsimd.dma_start(out=beta_b, in_=bcast_ap(beta_d.ap()))

        scr_z = dram.tile([1, NLOC], f32, tag="scr_z")
        scr_w0 = dram.tile([1, NLOC], f32, tag="scr_w0")

        # token n <-> (p, c) mapping is n = 8p + c (partition-major loads
        # give contiguous 16KB DMA descriptors); the output write uses the
        # same mapping, so results land in canonical row order.
        out_r = out_d.ap().rearrange("(p c) e -> p c e", c=8)

        # ---- setup phase (own pools, freed before the main-loop PSUM pools)
        sctx = ctx.enter_context(ExitStack())
        stage = sctx.enter_context(tc.tile_pool(name="stage", bufs=1))
        ps_setup = sctx.enter_context(tc.tile_pool(name="ps_set", bufs=2, space="PSUM"))

        # ---- loads. scalar HWDGE: wq, v, obs even groups; sync HWDGE:
        # wk, wv, obs odd groups. All with large per-partition descriptors.
        wq_f = stage.tile([P, 4, E], f32, tag="wq_f")
        nc.scalar.dma_start(wq_f, wq_d.ap().rearrange("(p c) e -> p c e", c=4))
        v_r = v_d.ap().rearrange("(p c) e -> p c e", c=8)
        nc.scalar.dma_start(v_f32, v_r)
        wk_f = stage.tile([P, 4, E], f32, tag="wk_f")
        nc.sync.dma_start(wk_f, wk_d.ap().rearrange("(p c) e -> p c e", c=4))
        wv_f = stage.tile([P, 4, E], f32, tag="wv_f")
        nc.sync.dma_start(wv_f, wv_d.ap().rearrange("(c p) e -> p c e", p=P))

        obs_r = obs_d.ap().rearrange("(p c) e -> p c e", c=32)
        obs_stage = []
        for g in range(8):
            of = ofp.tile([P, 4, E], f32, tag="obs_f")
            eng = nc.scalar if g % 2 == 0 else nc.sync
            eng.dma_start(of, obs_r[:, g * 4 : (g + 1) * 4, :])
            obs_stage.append(of)

        # ---- A = Wq.T @ Wk in fp8 DoubleRow (scale folded into exp)
        # (casts on DVE so ScalarE's act-table loads don't delay A)
        wq8 = stage.tile([P, 4, E], f8, tag="wq8")
        nc.vector.tensor_copy(wq8, wq_f)
        wk8 = stage.tile([P, 4, E], f8, tag="wk8")
        nc.vector.tensor_copy(wk8, wk_f)
        for ic in range(4):
            psA = ps_setup.tile([P, E], f32, tag="psA")
            for u in range(2):
                nc.tensor.matmul(
                    psA,
                    lhsT=wq8[:, 2 * u : 2 * u + 2, ic * P : (ic + 1) * P],
                    rhs=wk8[:, 2 * u : 2 * u + 2, :],
                    start=(u == 0),
                    stop=(u == 1),
                    perf_mode=DR,
                )
            nc.scalar.copy(A_sb[:, ic, :], psA)

        # ---- vT via TensorE transposes (f32 in, cast to f8 on copy-out)
        for ec in range(4):
            for g in range(2):
                pst = ps_setup.tile([P, 4 * P], f32, tag="pst")
                for j in range(4):
                    nk = g * 4 + j
                    nc.tensor.transpose(
                        pst[:, j * P : (j + 1) * P],
                        v_f32[:, nk, ec * P : (ec + 1) * P],
                        identity,
                    )
                nc.vector.tensor_copy(vT[:, ec, g * 512 : (g + 1) * 512], pst)

        # ---- vAT = (v @ A).T   [e2, n]  (fp8)
        for e2 in range(4):
            for nb in range(2):
                psv = ps_setup.tile([P, 512], f32, tag="psv")
                for u in range(2):
                    nc.tensor.matmul(
                        psv,
                        lhsT=A_sb[:, 2 * u : 2 * u + 2, e2 * P : (e2 + 1) * P],
                        rhs=vT[:, 2 * u : 2 * u + 2, nb * 512 : (nb + 1) * 512],
                        start=(u == 0),
                        stop=(u == 1),
                        perf_mode=DR,
                    )
                nc.vector.tensor_copy(vAT[:, e2, nb * 512 : (nb + 1) * 512], psv)

        # ---- WvT via TensorE transposes (only needed by the epilogue)
        for jc in range(4):  # e_in slice -> WvT partition chunk
            pst = ps_setup.tile([P, 4 * P], f32, tag="pst")
            for ic in range(4):  # e_out chunk
                nc.tensor.transpose(
                    pst[:, ic * P : (ic + 1) * P],
                    wv_f[:, ic, jc * P : (jc + 1) * P],
                    identity,
                )
            nc.scalar.copy(WvT[:, jc, :], pst)

        # ---- self score (token-major [n,1] per chunk) and w0 = exp(.-S)
        ps_sf = ps_setup.tile([P, 8], f32, tag="ps_sf")
        for ec in range(4):
            prod_ec = prodp.tile([P, NLOC], bf16, tag="prod")
            nc.vector.tensor_mul(prod_ec, vAT[:, ec, :], vT[:, ec, :])
            for nk in range(8):
                nc.tensor.matmul(
                    ps_sf[:, nk : nk + 1],
                    lhsT=prod_ec[:, nk * P : (nk + 1) * P],
                    rhs=ones_bf,
                    start=(ec == 0),
                    stop=(ec == 3),
                )
        nc.scalar.activation(
            w0, ps_sf, AF.Exp, bias=nshift_t, scale=1.0 / TEMPERATURE
        )
        # w0 row-major broadcast [P, NLOC] via DRAM roundtrip (GpSimd)
        nc.gpsimd.dma_start(scr_w0.rearrange("o (a p) -> (o p) a", p=P), w0)
        nc.gpsimd.dma_start(w0_bc, bcast_ap(scr_w0[:]))

        # ---- obs pipeline: f32 -> bf16 cast -> DMA XBAR transpose -> f8.
        # ScalarE primes groups 0-1 (pipeline start) and does all m-major f8
        # casts; DVE covers the rest. Transposes alternate HWDGE queues.
        for g in range(8):
            of = obs_stage[g]
            nc.scalar.copy(obs_f8[:, g * 4 : (g + 1) * 4, :], of)
            ob = obp.tile([P, 4, E], bf16, tag="obs_bf")
            if g < 2:
                nc.scalar.copy(ob, of)
            else:
                nc.vector.tensor_copy(ob, of)
            otb = otbp.tile([P, 4, 4, P], bf16, tag="obsT_bf")
            teng = nc.sync if g % 2 == 0 else nc.scalar
            teng.dma_start(otb, ob, transpose=True)
            # cast to f8 (dense 16-bit read)
            if g < 2:
                nc.scalar.copy(obsT[:, g * 4 : (g + 1) * 4, :, :], otb)
            else:
                nc.vector.tensor_copy(obsT[:, g * 4 : (g + 1) * 4, :, :], otb)

        # ---- w0 * v.T term, precomputed for the uT drains
        for ec in range(4):
            nc.vector.tensor_mul(w0v[:, ec, :], vT[:, ec, :], w0_bc)

        sctx.close()
        ps_s_pool = ctx.enter_context(tc.tile_pool(name="ps_s", bufs=1, space="PSUM"))
        ps_ut_pool = ctx.enter_context(tc.tile_pool(name="ps_ut", bufs=1, space="PSUM"))
        ps_z_pool = ctx.enter_context(tc.tile_pool(name="ps_z", bufs=1, space="PSUM"))
        ps_y_pool = ctx.enter_context(tc.tile_pool(name="ps_y", bufs=1, space="PSUM"))

        # ---- main loop: 2 token blocks of 512; 16 obs chunks of 256 each.
        # Emit score(t) then Z/uT(t-1): TensorE does uT work while ScalarE
        # runs exp(t), and ps_s (single-buffered, 2 banks) frees in time.
        def make_epilogue_chunks(nb):
            # returns 4 closures, one per token chunk; first also emits the
            # recipZ prep. ps_y is drained to SBUF by a fast scalar copy so
            # TensorE never waits on the DVE LayerNorm chain.
            def chunk(i):
                def emit():
                    nk = nb * 4 + i
                    if i == 0:
                        c4 = slice(nb * 4, (nb + 1) * 4)
                        nc.vector.tensor_add(ztok[:, c4], ztok[:, c4], w0[:, c4])
                        nc.vector.reciprocal(recipZ[:, c4], ztok[:, c4])
                    ps_y = ps_y_pool.tile([P, E], f32, tag="y")
                    for u in range(2):
                        nc.tensor.matmul(
                            ps_y,
                            lhsT=uT[:, 2 * u : 2 * u + 2, nk * P : (nk + 1) * P],
                            rhs=WvT[:, 2 * u : 2 * u + 2, :],
                            start=(u == 0),
                            stop=(u == 1),
                            perf_mode=DR,
                        )
                    y_sb = epi.tile([P, E], f32, tag="y_sb")
                    nc.scalar.copy(y_sb, ps_y)
                    y2 = epi.tile([P, E], f32, tag="y2")
                    nc.vector.scalar_tensor_tensor(
                        y2,
                        in0=y_sb,
                        scalar=recipZ[:, nk : nk + 1],
                        in1=v_f32[:, nk, :],
                        op0=ALU.mult,
                        op1=ALU.add,
                    )
                    stats = epi.tile([P, 6], f32, tag="stats")
                    nc.vector.bn_stats(stats, y2)
                    mv = epi.tile([P, 2], f32, tag="mv")
                    nc.vector.bn_aggr(mv, stats)
                    std = epi.tile([P, 1], f32, tag="std")
                    nc.scalar.activation(std, mv[:, 1:2], AF.Sqrt, bias=eps_t)
                    rstd = epi.tile([P, 1], f32, tag="rstd")
                    nc.vector.reciprocal(rstd, std)
                    nc.vector.tensor_scalar(
                        y2,
                        in0=y2,
                        scalar1=mv[:, 0:1],
                        scalar2=rstd,
                        op0=ALU.subtract,
                        op1=ALU.mult,
                    )
                    nc.vector.tensor_mul(y2, y2, gamma_b)
                    nc.vector.tensor_add(y2, y2, beta_b)
                    nc.sync.dma_start(out_r[:, nk, :], y2)
                return emit
            return [chunk(i) for i in range(4)]

        pending_epilogue = None
        for nb in range(2):
            nsl = slice(nb * 512, (nb + 1) * 512)
            ps_uT = ps_ut_pool.tile([P, 4, 512], f32, tag="uT")
            ps_z = ps_z_pool.tile([P, 512], f32, tag="z")
            pend = None  # (t, ex2) whose Z/uT matmuls are not yet emitted

            def flush(t, ex2):
                nc.tensor.matmul(
                    ps_z,
                    lhsT=ones_f8z,
                    rhs=ex2,
                    start=(t == 0),
                    stop=(t == 15),
                    perf_mode=DR,
                )
                for es in range(4):
                    nc.tensor.matmul(
                        ps_uT[:, es, :],
                        lhsT=obs_f8[:, 2 * t : 2 * t + 2, es * P : (es + 1) * P],
                        rhs=ex2,
                        start=(t == 0),
                        stop=(t == 15),
                        perf_mode=DR,
                    )

            for t in range(16):
                ps_s = ps_s_pool.tile([P, 2, 512], f32, tag="s")
                for j in range(2):
                    mc = 2 * t + j
                    for u in range(2):
                        nc.tensor.matmul(
                            ps_s[:, j, :],
                            lhsT=obsT[:, mc, 2 * u : 2 * u + 2, :],
                            rhs=vAT[:, 2 * u : 2 * u + 2, nsl],
                            start=(u == 0),
                            stop=(u == 1),
                            perf_mode=DR,
                        )
                if pending_epilogue and t < 4:
                    # overlap block 0's epilogue with block 1's first steps
                    pending_epilogue[t]()
                if pend is not None:
                    flush(*pend)
                ex2 = expp.tile([P, 2, 512], f8, tag="ex")
                nc.scalar.activation(
                    ex2, ps_s, AF.Exp, bias=nshift_t, scale=1.0 / TEMPERATURE
                )
                pend = (t, ex2)
            flush(*pend)

            # drain Z first (epilogue roundtrip starts while uT drains)
            zrow = epi.tile([1, 512], f32, tag="zrow")
            nc.vector.tensor_copy(zrow, ps_z[0:1, :])
            nc.sync.dma_start(scr_z[:, nsl], zrow)
            nc.sync.dma_start(
                ztok[:, nb * 4 : (nb + 1) * 4],
                scr_z[:, nsl].rearrange("o (a p) -> (o p) a", p=P),
            )
            # drain uT (+ fold in w0 * v term)
            for ec in range(4):
                nc.vector.tensor_add(
                    uT[:, ec, nsl], w0v[:, ec, nsl], ps_uT[:, ec, :]
                )
            if nb == 0:
                pending_epilogue = make_epilogue_chunks(0)
            else:
                for step in make_epilogue_chunks(1):
                    step()

    nc.compile()
    return nc


def _get_nc():
    global _CACHED_NC
    if _CACHED_NC is None:
        _CACHED_NC = _build()
    return _CACHED_NC


def _in_maps(v_code, obs_code, Wq, Wk, Wv, gamma, beta):
    def f(x):
        return np.ascontiguousarray(np.asarray(x), dtype=np.float32)

    shared = {
        "obs_code": f(obs_code),
        "Wq": f(Wq),
        "Wk": f(Wk),
        "Wv": f(Wv),
        "gamma": f(gamma),
        "beta": f(beta),
    }
    return [
        {"v_code": f(v_code[c * NLOC : (c + 1) * NLOC]), **shared}
        for c in range(CORES)
    ]


def run(trace=False, **inputs):
    from concourse.bass_utils import run_bass_kernel_spmd

    nc = _get_nc()
    res = run_bass_kernel_spmd(
        nc, _in_maps(**inputs), core_ids=list(range(CORES)), trace=trace
    )
    out = np.concatenate(
        [res.results[c]["out"] for c in range(CORES)], axis=0
    ).astype(np.float32)
    return out, res


def kernel(**inputs) -> np.ndarray:
    out, _ = run(trace=False, **inputs)
    return out
